# revision 25
# baseline (speedup 1.0000x reference)
"""Multi-head attention (B=4, S=2048, D=1024, H=16) on 8 TRN2 NeuronCores.

Sharding: core c -> batch c//2, query-row half c%2, all 16 heads.
No collectives needed: each core owns a disjoint (batch, query-row) slice of
both outputs.

Per-core math (all matmuls bf16 with fp32 PSUM accumulation):
  QT = Wq^T-free:  psum[of,tok]  = sum_c Wq[c,of]^T-as-lhsT . XqT[c,tok]
  KT likewise; V computed in natural [tok, of] layout with lhsT = XvT.
  V is stored augmented with a ones-column per head (65 cols/head) so the
  attention-value matmul also produces the softmax denominator for free.
  scoresT[kt,qt] = KT_h^T . QT_h ; exp on ACT (scores are tiny, |s|<4,
  so no max-subtraction is needed); ctxT/rowsum accumulate over kt chunks;
  attn = exp * (1/rowsum) written f32; out = ctxT^T-as-lhsT . Wo + bo.

Host side only slices/transposes/casts inputs and reassembles outputs
(attn comes back as [H, kt, qt] per core and is swapped on the host).
"""

import os
import sys

for _p in ("/opt/trn_rl_repo",):
    if _p not in sys.path:
        sys.path.insert(0, _p)

import ml_dtypes
import numpy as np

import concourse.mybir as mybir
import concourse.tile as tile
from concourse import bacc, bass_utils
from concourse.bass_interp import get_hw_module

B, S, D, H, DK = 4, 2048, 1024, 16, 64
SCALE = 1.0 / float(np.sqrt(DK))
SQ = S // 2          # query rows per core
N_CORES = 8
BF16 = mybir.dt.bfloat16
F32 = mybir.dt.float32

LAST_EXEC_NS = None


def _emit(nc, tc, aps):
    from contextlib import ExitStack

    xq, xk, xv = aps["xqT"], aps["xkT"], aps["xvT"]
    wq, wk, wv, wo = aps["wq"], aps["wk"], aps["wv"], aps["wo"]
    bq, bk, bv, bo = aps["bq"], aps["bk"], aps["bv"], aps["bo"]
    attnT, outp = aps["attnT"], aps["outp"]

    Exp = mybir.ActivationFunctionType.Exp
    KC = D // 128                 # 8 contraction chunks
    QTB = SQ // 512               # 2 query-token blocks of 512
    KTT = S // 128                # 16 key-token tiles of 128

    persist = tc.tile_pool(name="persist", bufs=1)
    dram_qk = tc.tile_pool(name="dram_qk", bufs=1, space="DRAM")
    with persist as pp, dram_qk as dq:
        QTd = dq.tile([D, SQ], BF16, tag="QTd")   # Q^T in DRAM [of, qtok]
        KTd = dq.tile([D, S], BF16, tag="KTd")    # K^T in DRAM [of, ktok]
        VA = pp.tile([128, KTT, H * (DK + 1)], BF16, tag="VA")  # [ktok_p, ktc, h*65+dv]
        CT = pp.tile([128, KC, SQ], BF16, tag="CT")          # ctxT [feat_p, feat_c, qtok]
        bqs = pp.tile([128, KC], F32, tag="bqs")
        bks = pp.tile([128, KC], F32, tag="bks")
        nc.sync.dma_start(bqs[:], bq.rearrange("(c p) -> p c", p=128))
        nc.sync.dma_start(bks[:], bk.rearrange("(c p) -> p c", p=128))
        wo_sb = pp.tile([128, KC, D], BF16, tag="wo")
        nc.sync.dma_start(wo_sb[:], wo.rearrange("(c p) f -> p c f", p=128))
        bos = pp.tile([128, D], F32, tag="bos")
        nc.sync.dma_start(bos[:], bo.rearrange("(o f) -> o f", o=1)
                          .to_broadcast([128, D]))

        # ones columns of VA (slot 64 of each 65-wide head block)
        VA4 = VA.rearrange("p k (h c) -> p k h c", c=DK + 1)
        nc.vector.memset(VA4[:, :, :, DK], 1.0)

        # ---- Phase 1: projections ----
        with ExitStack() as ph1:
            wpool = ph1.enter_context(tc.tile_pool(name="w1", bufs=1))
            xpool = ph1.enter_context(tc.tile_pool(name="xs", bufs=3))
            ps1 = ph1.enter_context(tc.tile_pool(name="ps1", bufs=4, space="PSUM"))

            # bv varies along the free axis -> replicate across partitions via DMA
            bvs = wpool.tile([128, D], F32, tag="bvs")
            nc.sync.dma_start(bvs[:], bv.rearrange("(o f) -> o f", o=1)
                              .to_broadcast([128, D]))
            wq_sb = wpool.tile([128, KC, D], BF16, tag="wq")
            wk_sb = wpool.tile([128, KC, D], BF16, tag="wk")
            nc.sync.dma_start(wq_sb[:], wq.rearrange("(c p) f -> p c f", p=128))
            nc.sync.dma_start(wk_sb[:], wk.rearrange("(c p) f -> p c f", p=128))

            xq_t = xq.rearrange("(c p) t -> p c t", p=128)
            xk_t = xk.rearrange("(c p) t -> p c t", p=128)
            xv_t = xv.rearrange("(c p) t -> p c t", p=128)

            spool = ph1.enter_context(tc.tile_pool(name="stage", bufs=4))
            # QT[of, qtok] -> DRAM
            for tb in range(QTB):
                x = xpool.tile([128, KC, 512], BF16, tag="xq")
                nc.sync.dma_start(x[:], xq_t[:, :, tb * 512:(tb + 1) * 512])
                for ot in range(KC):
                    ps = ps1.tile([128, 512], F32, tag="p1")
                    for c in range(KC):
                        nc.tensor.matmul(
                            ps[:], lhsT=wq_sb[:, c, ot * 128:(ot + 1) * 128],
                            rhs=x[:, c, :], start=(c == 0), stop=(c == KC - 1))
                    st = spool.tile([128, 512], BF16, tag="st")
                    nc.scalar.add(st[:], ps[:], bqs[:, ot:ot + 1])
                    nc.sync.dma_start(
                        QTd[ot * 128:(ot + 1) * 128, tb * 512:(tb + 1) * 512],
                        st[:])
            # KT[of, ktok] -> DRAM
            for tb in range(S // 512):
                x = xpool.tile([128, KC, 512], BF16, tag="xq")
                nc.sync.dma_start(x[:], xk_t[:, :, tb * 512:(tb + 1) * 512])
                for ot in range(KC):
                    ps = ps1.tile([128, 512], F32, tag="p1")
                    for c in range(KC):
                        nc.tensor.matmul(
                            ps[:], lhsT=wk_sb[:, c, ot * 128:(ot + 1) * 128],
                            rhs=x[:, c, :], start=(c == 0), stop=(c == KC - 1))
                    st = spool.tile([128, 512], BF16, tag="st")
                    nc.scalar.add(st[:], ps[:], bks[:, ot:ot + 1])
                    nc.sync.dma_start(
                        KTd[ot * 128:(ot + 1) * 128, tb * 512:(tb + 1) * 512],
                        st[:])
            # V natural [ktok, of] into augmented layout
            wv_sb = wpool.tile([128, KC, D], BF16, tag="wv")
            nc.sync.dma_start(wv_sb[:], wv.rearrange("(c p) f -> p c f", p=128))
            for tt in range(KTT):
                x = xpool.tile([128, KC, 128], BF16, tag="xv")
                nc.sync.dma_start(x[:], xv_t[:, :, tt * 128:(tt + 1) * 128])
                for ob in range(2):
                    ps = ps1.tile([128, 512], F32, tag="p1")
                    for c in range(KC):
                        nc.tensor.matmul(
                            ps[:], lhsT=x[:, c, :],
                            rhs=wv_sb[:, c, ob * 512:(ob + 1) * 512],
                            start=(c == 0), stop=(c == KC - 1))
                    nc.vector.tensor_add(
                        out=VA4[:, tt, ob * 8:(ob + 1) * 8, 0:DK],
                        in0=ps.rearrange("p (h c) -> p h c", c=DK),
                        in1=bvs[:, ob * 512:(ob + 1) * 512]
                        .rearrange("p (h c) -> p h c", c=DK))

        # ---- Phase 2: attention ----
        with ExitStack() as ph2:
            epool = ph2.enter_context(tc.tile_pool(name="expp", bufs=42))
            kqpool = ph2.enter_context(tc.tile_pool(name="kq", bufs=2))
            apool = ph2.enter_context(tc.tile_pool(name="attnf", bufs=4))
            rpool = ph2.enter_context(tc.tile_pool(name="recip", bufs=2))
            dpool = ph2.enter_context(tc.tile_pool(name="rdram", bufs=2, space="DRAM"))
            ps_sc = ph2.enter_context(tc.tile_pool(name="ps_sc", bufs=2, space="PSUM"))
            ps_cx = ph2.enter_context(tc.tile_pool(name="ps_cx", bufs=2, space="PSUM"))

            Ln = mybir.ActivationFunctionType.Ln
            GP_KTC = {1, 4, 7, 10, 13, 15}   # normalize tiles offloaded to GpSimd
            for hp in range(H // 2):
                heads = (2 * hp, 2 * hp + 1)
                # stream this pair's K^T/Q^T rows (both heads, partitions
                # 0-63 = even head, 64-127 = odd head) from DRAM
                ktp = kqpool.tile([128, KTT, 128], BF16, tag="ktp")
                qtp = kqpool.tile([128, SQ], BF16, tag="qtp")
                nc.sync.dma_start(
                    ktp[:], KTd[hp * 128:(hp + 1) * 128, :]
                    .rearrange("p (k j) -> p k j", j=128))
                nc.sync.dma_start(qtp[:], QTd[hp * 128:(hp + 1) * 128, :])
                et = {h: [None] * KTT for h in heads}
                cp = {h: ps_cx.tile([DK + 1, SQ], F32, tag="cp",
                                    name=f"cp_{h}") for h in heads}
                # software pipeline: AV matmuls trail the scores/exp stream
                # by AVL iterations, so the PE's in-order queue has ready
                # scores work ahead of the first AV (which blocks on the
                # previous pair's cp PSUM release)
                AVL = 4
                for step in range(KTT + AVL):
                    if step < KTT:
                        ktc = step
                        sp = {}
                        # explicit tile_position packs the two K=64 matmuls
                        # into disjoint PE row groups -> they run concurrently
                        for qtb in range(QTB):
                            for h in heads:
                                base = (h % 2) * 64
                                if h not in sp:
                                    sp[h] = ps_sc.tile([128, SQ], F32,
                                                       tag="sp",
                                                       name=f"sp_{h}_{ktc}")
                                nc.tensor.matmul(
                                    sp[h][:, qtb * 512:(qtb + 1) * 512],
                                    lhsT=ktp[base:base + 64, ktc, :],
                                    rhs=qtp[base:base + 64,
                                            qtb * 512:(qtb + 1) * 512],
                                    start=True, stop=True,
                                    tile_position=(base, 0))
                        for h in heads:
                            e = epool.tile([128, SQ], BF16, tag="et",
                                           name=f"et_{h}_{ktc}")
                            et[h][ktc] = e
                            nc.scalar.activation(e[:], sp[h][:], Exp,
                                                 scale=SCALE)
                    if step >= AVL:
                        k2 = step - AVL
                        for qtb in range(QTB):
                            for h in heads:
                                nc.tensor.matmul(
                                    cp[h][:, qtb * 512:(qtb + 1) * 512],
                                    lhsT=VA[:, k2,
                                            h * (DK + 1):(h + 1) * (DK + 1)],
                                    rhs=et[h][k2][:,
                                                  qtb * 512:(qtb + 1) * 512],
                                    start=(k2 == 0), stop=(k2 == KTT - 1))
                for h in heads:
                    base, ch = (h % 2) * 64, h // 2
                    rcb = rpool.tile([128, SQ], F32, tag="rcb")
                    # 1/rowsum = exp(-ln(rowsum)) on ACT (cheaper than DVE
                    # InstReciprocal and off the DVE critical path)
                    nc.scalar.activation(rcb[0:1, :], cp[h][DK:DK + 1, :], Ln)
                    nc.scalar.activation(rcb[0:1, :], rcb[0:1, :], Exp,
                                         scale=-1.0)
                    rcd = dpool.tile([1, SQ], F32, tag="rcd")
                    nc.sync.dma_start(rcd[:], rcb[0:1, :])
                    nc.sync.dma_start(rcb[:], rcd.to_broadcast([128, SQ]))
                    nc.vector.tensor_mul(
                        out=CT[base:base + 64, ch, :], in0=cp[h][0:DK, :],
                        in1=rcb[0:DK, :])
                    for ktc in range(KTT):
                        af = apool.tile([128, SQ], F32, tag="af")
                        eng = nc.gpsimd if ktc in GP_KTC else nc.vector
                        eng.tensor_mul(out=af[:], in0=et[h][ktc][:], in1=rcb[:])
                        nc.sync.dma_start(attnT[h, ktc * 128:(ktc + 1) * 128, :],
                                          af[:])

        # ---- Phase 3: output projection ----
        with ExitStack() as ph3:
            opool = ph3.enter_context(tc.tile_pool(name="ob", bufs=3))
            ps3 = ph3.enter_context(tc.tile_pool(name="ps3", bufs=4, space="PSUM"))
            for tt in range(SQ // 128):
                for ob in range(2):
                    ps = ps3.tile([128, 512], F32, tag="p3")
                    for c in range(KC):
                        nc.tensor.matmul(
                            ps[:], lhsT=CT[:, c, tt * 128:(tt + 1) * 128],
                            rhs=wo_sb[:, c, ob * 512:(ob + 1) * 512],
                            start=(c == 0), stop=(c == KC - 1))
                    obt = opool.tile([128, 512], F32, tag="obt")
                    nc.vector.tensor_add(
                        out=obt[:], in0=ps[:],
                        in1=bos[:, ob * 512:(ob + 1) * 512])
                    nc.sync.dma_start(
                        outp[tt * 128:(tt + 1) * 128, ob * 512:(ob + 1) * 512], obt[:])


def _build():
    nc = bacc.Bacc("TRN2", target_bir_lowering=False, debug=False,
                   num_devices=N_CORES)
    aps = {}
    for name, shape, dt in [
        ("xqT", (D, SQ), BF16), ("xkT", (D, S), BF16), ("xvT", (D, S), BF16),
        ("wq", (D, D), BF16), ("wk", (D, D), BF16), ("wv", (D, D), BF16),
        ("wo", (D, D), BF16),
        ("bq", (D,), F32), ("bk", (D,), F32), ("bv", (D,), F32), ("bo", (D,), F32),
    ]:
        aps[name] = nc.dram_tensor(name, shape, dt, kind="ExternalInput").ap()
    aps["attnT"] = nc.dram_tensor("attnT", (H, S, SQ), F32,
                                  kind="ExternalOutput").ap()
    aps["outp"] = nc.dram_tensor("outp", (SQ, D), F32, kind="ExternalOutput").ap()

    with tile.TileContext(nc) as tc:
        _emit(nc, tc, aps)
    nc.compile()
    nc.m = get_hw_module(nc.m)
    return nc


_NC = None


def kernel(query, key_, value, Wq, bq, Wk, bk, Wv, bv, Wo, bo):
    global _NC, LAST_EXEC_NS
    if _NC is None:
        _NC = _build()

    bf = ml_dtypes.bfloat16
    query = np.asarray(query, np.float32)
    key_ = np.asarray(key_, np.float32)
    value = np.asarray(value, np.float32)
    wqs = np.ascontiguousarray(np.asarray(Wq, np.float32).astype(bf))
    wks = np.ascontiguousarray(np.asarray(Wk, np.float32).astype(bf))
    wvs = np.ascontiguousarray(np.asarray(Wv, np.float32).astype(bf))
    wos = np.ascontiguousarray(np.asarray(Wo, np.float32).astype(bf))
    biases = {n: np.asarray(b, np.float32) for n, b in
              [("bq", bq), ("bk", bk), ("bv", bv), ("bo", bo)]}

    in_maps = []
    for c in range(N_CORES):
        b, half = c // 2, c % 2
        qs = slice(half * SQ, (half + 1) * SQ)
        in_maps.append({
            "xqT": np.ascontiguousarray(query[b, qs, :].T.astype(bf)),
            "xkT": np.ascontiguousarray(key_[b].T.astype(bf)),
            "xvT": np.ascontiguousarray(value[b].T.astype(bf)),
            "wq": wqs, "wk": wks, "wv": wvs, "wo": wos,
            **biases,
        })

    kw = {}
    tdir = os.environ.get("MHA_TRACE_DIR")
    if tdir:
        os.makedirs(tdir, exist_ok=True)
        kw["tmpdir"] = tdir
    res = bass_utils.run_bass_kernel_spmd(_NC, in_maps,
                                          core_ids=list(range(N_CORES)), **kw)
    LAST_EXEC_NS = res.exec_time_ns

    out = np.empty((B, S, D), np.float32)
    attn = np.empty((B, H, S, S), np.float32)
    for c in range(N_CORES):
        r = res.results[c]
        b, half = c // 2, c % 2
        qs = slice(half * SQ, (half + 1) * SQ)
        out[b, qs, :] = r["outp"]
        attn[b, :, qs, :] = np.swapaxes(r["attnT"], 1, 2)
    return out, attn


# revision 26
# speedup vs baseline: 1.0585x; 1.0585x over previous
"""Multi-head attention (B=4, S=2048, D=1024, H=16) on 8 TRN2 NeuronCores.

Sharding: core c -> batch c//2, query-row half c%2, all 16 heads.
No collectives needed: each core owns a disjoint (batch, query-row) slice of
both outputs.

Per-core math (all matmuls bf16 with fp32 PSUM accumulation):
  QT = Wq^T-free:  psum[of,tok]  = sum_c Wq[c,of]^T-as-lhsT . XqT[c,tok]
  KT likewise; V computed in natural [tok, of] layout with lhsT = XvT.
  V is stored augmented with a ones-column per head (65 cols/head) so the
  attention-value matmul also produces the softmax denominator for free.
  scoresT[kt,qt] = KT_h^T . QT_h ; exp on ACT (scores are tiny, |s|<4,
  so no max-subtraction is needed); ctxT/rowsum accumulate over kt chunks;
  attn = exp * (1/rowsum) written f32; out = ctxT^T-as-lhsT . Wo + bo.

Host side only slices/transposes/casts inputs and reassembles outputs
(attn comes back as [H, kt, qt] per core and is swapped on the host).
"""

import os
import sys

for _p in ("/opt/trn_rl_repo",):
    if _p not in sys.path:
        sys.path.insert(0, _p)

import ml_dtypes
import numpy as np

import concourse.mybir as mybir
import concourse.tile as tile
from concourse import bacc, bass_utils
from concourse.bass_interp import get_hw_module

B, S, D, H, DK = 4, 2048, 1024, 16, 64
SCALE = 1.0 / float(np.sqrt(DK))
SQ = S // 2          # query rows per core
N_CORES = 8
BF16 = mybir.dt.bfloat16
F32 = mybir.dt.float32

LAST_EXEC_NS = None


def _emit(nc, tc, aps):
    from contextlib import ExitStack

    xq, xk, xv = aps["xqT"], aps["xkT"], aps["xvT"]
    wq, wk, wv, wo = aps["wq"], aps["wk"], aps["wv"], aps["wo"]
    bq, bk, bv, bo = aps["bq"], aps["bk"], aps["bv"], aps["bo"]
    attnT, outp = aps["attnT"], aps["outp"]

    Exp = mybir.ActivationFunctionType.Exp
    KC = D // 128                 # 8 contraction chunks
    QTB = SQ // 512               # 2 query-token blocks of 512
    KTT = S // 128                # 16 key-token tiles of 128

    persist = tc.tile_pool(name="persist", bufs=1)
    dram_qk = tc.tile_pool(name="dram_qk", bufs=1, space="DRAM")
    with persist as pp, dram_qk as dq:
        QTd = dq.tile([D, SQ], BF16, tag="QTd")   # Q^T in DRAM [of, qtok]
        KTd = dq.tile([D, S], BF16, tag="KTd")    # K^T in DRAM [of, ktok]
        VA = pp.tile([128, KTT, H * (DK + 1)], BF16, tag="VA")  # [ktok_p, ktc, h*65+dv]
        CT = pp.tile([128, KC, SQ], BF16, tag="CT")          # ctxT [feat_p, feat_c, qtok]
        bqs = pp.tile([128, KC], F32, tag="bqs")
        bks = pp.tile([128, KC], F32, tag="bks")
        nc.sync.dma_start(bqs[:], bq.rearrange("(c p) -> p c", p=128))
        nc.sync.dma_start(bks[:], bk.rearrange("(c p) -> p c", p=128))
        wo_sb = pp.tile([128, KC, D], BF16, tag="wo")
        nc.sync.dma_start(wo_sb[:], wo.rearrange("(c p) f -> p c f", p=128))
        bos = pp.tile([128, D], F32, tag="bos")
        nc.sync.dma_start(bos[:], bo.rearrange("(o f) -> o f", o=1)
                          .to_broadcast([128, D]))

        # ones columns of VA (slot 64 of each 65-wide head block)
        VA4 = VA.rearrange("p k (h c) -> p k h c", c=DK + 1)
        nc.vector.memset(VA4[:, :, :, DK], 1.0)

        # ---- Phase 1: projections ----
        with ExitStack() as ph1:
            wpool = ph1.enter_context(tc.tile_pool(name="w1", bufs=1))
            xpool = ph1.enter_context(tc.tile_pool(name="xs", bufs=3))
            ps1 = ph1.enter_context(tc.tile_pool(name="ps1", bufs=4, space="PSUM"))

            # bv varies along the free axis -> replicate across partitions via DMA
            bvs = wpool.tile([128, D], F32, tag="bvs")
            nc.sync.dma_start(bvs[:], bv.rearrange("(o f) -> o f", o=1)
                              .to_broadcast([128, D]))
            wq_sb = wpool.tile([128, KC, D], BF16, tag="wq")
            wk_sb = wpool.tile([128, KC, D], BF16, tag="wk")
            nc.sync.dma_start(wq_sb[:], wq.rearrange("(c p) f -> p c f", p=128))
            nc.sync.dma_start(wk_sb[:], wk.rearrange("(c p) f -> p c f", p=128))

            xq_t = xq.rearrange("(c p) t -> p c t", p=128)
            xk_t = xk.rearrange("(c p) t -> p c t", p=128)
            xv_t = xv.rearrange("(c p) t -> p c t", p=128)

            spool = ph1.enter_context(tc.tile_pool(name="stage", bufs=4))
            # QT[of, qtok] -> DRAM
            for tb in range(QTB):
                x = xpool.tile([128, KC, 512], BF16, tag="xq")
                nc.sync.dma_start(x[:], xq_t[:, :, tb * 512:(tb + 1) * 512])
                for ot in range(KC):
                    ps = ps1.tile([128, 512], F32, tag="p1")
                    for c in range(KC):
                        nc.tensor.matmul(
                            ps[:], lhsT=wq_sb[:, c, ot * 128:(ot + 1) * 128],
                            rhs=x[:, c, :], start=(c == 0), stop=(c == KC - 1))
                    st = spool.tile([128, 512], BF16, tag="st")
                    nc.scalar.add(st[:], ps[:], bqs[:, ot:ot + 1])
                    nc.sync.dma_start(
                        QTd[ot * 128:(ot + 1) * 128, tb * 512:(tb + 1) * 512],
                        st[:])
            # KT[of, ktok] -> DRAM
            for tb in range(S // 512):
                x = xpool.tile([128, KC, 512], BF16, tag="xq")
                nc.sync.dma_start(x[:], xk_t[:, :, tb * 512:(tb + 1) * 512])
                for ot in range(KC):
                    ps = ps1.tile([128, 512], F32, tag="p1")
                    for c in range(KC):
                        nc.tensor.matmul(
                            ps[:], lhsT=wk_sb[:, c, ot * 128:(ot + 1) * 128],
                            rhs=x[:, c, :], start=(c == 0), stop=(c == KC - 1))
                    st = spool.tile([128, 512], BF16, tag="st")
                    nc.scalar.add(st[:], ps[:], bks[:, ot:ot + 1])
                    nc.sync.dma_start(
                        KTd[ot * 128:(ot + 1) * 128, tb * 512:(tb + 1) * 512],
                        st[:])
            # V natural [ktok, of] into augmented layout
            wv_sb = wpool.tile([128, KC, D], BF16, tag="wv")
            nc.sync.dma_start(wv_sb[:], wv.rearrange("(c p) f -> p c f", p=128))
            for tt in range(KTT):
                x = xpool.tile([128, KC, 128], BF16, tag="xv")
                nc.sync.dma_start(x[:], xv_t[:, :, tt * 128:(tt + 1) * 128])
                for ob in range(2):
                    ps = ps1.tile([128, 512], F32, tag="p1")
                    for c in range(KC):
                        nc.tensor.matmul(
                            ps[:], lhsT=x[:, c, :],
                            rhs=wv_sb[:, c, ob * 512:(ob + 1) * 512],
                            start=(c == 0), stop=(c == KC - 1))
                    nc.vector.tensor_add(
                        out=VA4[:, tt, ob * 8:(ob + 1) * 8, 0:DK],
                        in0=ps.rearrange("p (h c) -> p h c", c=DK),
                        in1=bvs[:, ob * 512:(ob + 1) * 512]
                        .rearrange("p (h c) -> p h c", c=DK))

        # ---- Phase 2: attention ----
        with ExitStack() as ph2:
            epool = ph2.enter_context(tc.tile_pool(name="expp", bufs=42))
            kqpool = ph2.enter_context(tc.tile_pool(name="kq", bufs=2))
            apool = ph2.enter_context(tc.tile_pool(name="attnf", bufs=4))
            rpool = ph2.enter_context(tc.tile_pool(name="recip", bufs=2))
            dpool = ph2.enter_context(tc.tile_pool(name="rdram", bufs=2, space="DRAM"))
            ps_sc = ph2.enter_context(tc.tile_pool(name="ps_sc", bufs=2, space="PSUM"))
            ps_cx = ph2.enter_context(tc.tile_pool(name="ps_cx", bufs=2, space="PSUM"))

            Ln = mybir.ActivationFunctionType.Ln
            GP_KTC = {1, 4, 7, 10, 13, 15}   # normalize tiles offloaded to GpSimd
            for hp in range(H // 2):
                heads = (2 * hp, 2 * hp + 1)
                # stream this pair's K^T/Q^T rows (both heads, partitions
                # 0-63 = even head, 64-127 = odd head) from DRAM
                ktp = kqpool.tile([128, KTT, 128], BF16, tag="ktp")
                qtp = kqpool.tile([128, SQ], BF16, tag="qtp")
                nc.sync.dma_start(
                    ktp[:], KTd[hp * 128:(hp + 1) * 128, :]
                    .rearrange("p (k j) -> p k j", j=128))
                nc.sync.dma_start(qtp[:], QTd[hp * 128:(hp + 1) * 128, :])
                et = {h: [None] * KTT for h in heads}
                cp = {h: ps_cx.tile([DK + 1, SQ], F32, tag="cp",
                                    name=f"cp_{h}") for h in heads}
                for ktc in range(KTT):
                    sp = {}
                    # explicit tile_position packs the two K=64 matmuls into
                    # disjoint PE row groups -> they run concurrently
                    for qtb in range(QTB):
                        for h in heads:
                            base = (h % 2) * 64
                            if h not in sp:
                                sp[h] = ps_sc.tile([128, SQ], F32, tag="sp",
                                                   name=f"sp_{h}_{ktc}")
                            nc.tensor.matmul(
                                sp[h][:, qtb * 512:(qtb + 1) * 512],
                                lhsT=ktp[base:base + 64, ktc, :],
                                rhs=qtp[base:base + 64,
                                        qtb * 512:(qtb + 1) * 512],
                                start=True, stop=True,
                                tile_position=(base, 0))
                    for h in heads:
                        e = epool.tile([128, SQ], BF16, tag="et",
                                       name=f"et_{h}_{ktc}")
                        et[h][ktc] = e
                        nc.scalar.activation(e[:], sp[h][:], Exp, scale=SCALE)
                    for qtb in range(QTB):
                        for h in heads:
                            nc.tensor.matmul(
                                cp[h][:, qtb * 512:(qtb + 1) * 512],
                                lhsT=VA[:, ktc, h * (DK + 1):(h + 1) * (DK + 1)],
                                rhs=et[h][ktc][:, qtb * 512:(qtb + 1) * 512],
                                start=(ktc == 0), stop=(ktc == KTT - 1))
                for h in heads:
                    base, ch = (h % 2) * 64, h // 2
                    rcb = rpool.tile([128, SQ], F32, tag="rcb")
                    # 1/rowsum = exp(-ln(rowsum)) on ACT (cheaper than DVE
                    # InstReciprocal and off the DVE critical path)
                    nc.scalar.activation(rcb[0:1, :], cp[h][DK:DK + 1, :], Ln)
                    nc.scalar.activation(rcb[0:1, :], rcb[0:1, :], Exp,
                                         scale=-1.0)
                    rcd = dpool.tile([1, SQ], F32, tag="rcd")
                    nc.sync.dma_start(rcd[:], rcb[0:1, :])
                    nc.sync.dma_start(rcb[:], rcd.to_broadcast([128, SQ]))
                    nc.vector.tensor_mul(
                        out=CT[base:base + 64, ch, :], in0=cp[h][0:DK, :],
                        in1=rcb[0:DK, :])
                    for ktc in range(KTT):
                        af = apool.tile([128, SQ], F32, tag="af")
                        eng = nc.gpsimd if ktc in GP_KTC else nc.vector
                        eng.tensor_mul(out=af[:], in0=et[h][ktc][:], in1=rcb[:])
                        nc.sync.dma_start(attnT[h, ktc * 128:(ktc + 1) * 128, :],
                                          af[:])

        # ---- Phase 3: output projection ----
        with ExitStack() as ph3:
            opool = ph3.enter_context(tc.tile_pool(name="ob", bufs=3))
            ps3 = ph3.enter_context(tc.tile_pool(name="ps3", bufs=4, space="PSUM"))
            for tt in range(SQ // 128):
                for ob in range(2):
                    ps = ps3.tile([128, 512], F32, tag="p3")
                    for c in range(KC):
                        nc.tensor.matmul(
                            ps[:], lhsT=CT[:, c, tt * 128:(tt + 1) * 128],
                            rhs=wo_sb[:, c, ob * 512:(ob + 1) * 512],
                            start=(c == 0), stop=(c == KC - 1))
                    obt = opool.tile([128, 512], F32, tag="obt")
                    nc.vector.tensor_add(
                        out=obt[:], in0=ps[:],
                        in1=bos[:, ob * 512:(ob + 1) * 512])
                    nc.sync.dma_start(
                        outp[tt * 128:(tt + 1) * 128, ob * 512:(ob + 1) * 512], obt[:])


def _build():
    nc = bacc.Bacc("TRN2", target_bir_lowering=False, debug=False,
                   num_devices=N_CORES)
    aps = {}
    for name, shape, dt in [
        ("xqT", (D, SQ), BF16), ("xkT", (D, S), BF16), ("xvT", (D, S), BF16),
        ("wq", (D, D), BF16), ("wk", (D, D), BF16), ("wv", (D, D), BF16),
        ("wo", (D, D), BF16),
        ("bq", (D,), F32), ("bk", (D,), F32), ("bv", (D,), F32), ("bo", (D,), F32),
    ]:
        aps[name] = nc.dram_tensor(name, shape, dt, kind="ExternalInput").ap()
    aps["attnT"] = nc.dram_tensor("attnT", (H, S, SQ), F32,
                                  kind="ExternalOutput").ap()
    aps["outp"] = nc.dram_tensor("outp", (SQ, D), F32, kind="ExternalOutput").ap()

    with tile.TileContext(nc) as tc:
        _emit(nc, tc, aps)
    nc.compile()
    nc.m = get_hw_module(nc.m)
    return nc


_NC = None


def kernel(query, key_, value, Wq, bq, Wk, bk, Wv, bv, Wo, bo):
    global _NC, LAST_EXEC_NS
    if _NC is None:
        _NC = _build()

    bf = ml_dtypes.bfloat16
    query = np.asarray(query, np.float32)
    key_ = np.asarray(key_, np.float32)
    value = np.asarray(value, np.float32)
    wqs = np.ascontiguousarray(np.asarray(Wq, np.float32).astype(bf))
    wks = np.ascontiguousarray(np.asarray(Wk, np.float32).astype(bf))
    wvs = np.ascontiguousarray(np.asarray(Wv, np.float32).astype(bf))
    wos = np.ascontiguousarray(np.asarray(Wo, np.float32).astype(bf))
    biases = {n: np.asarray(b, np.float32) for n, b in
              [("bq", bq), ("bk", bk), ("bv", bv), ("bo", bo)]}

    in_maps = []
    for c in range(N_CORES):
        b, half = c // 2, c % 2
        qs = slice(half * SQ, (half + 1) * SQ)
        in_maps.append({
            "xqT": np.ascontiguousarray(query[b, qs, :].T.astype(bf)),
            "xkT": np.ascontiguousarray(key_[b].T.astype(bf)),
            "xvT": np.ascontiguousarray(value[b].T.astype(bf)),
            "wq": wqs, "wk": wks, "wv": wvs, "wo": wos,
            **biases,
        })

    kw = {}
    tdir = os.environ.get("MHA_TRACE_DIR")
    if tdir:
        os.makedirs(tdir, exist_ok=True)
        kw["tmpdir"] = tdir
    res = bass_utils.run_bass_kernel_spmd(_NC, in_maps,
                                          core_ids=list(range(N_CORES)), **kw)
    LAST_EXEC_NS = res.exec_time_ns

    out = np.empty((B, S, D), np.float32)
    attn = np.empty((B, H, S, S), np.float32)
    for c in range(N_CORES):
        r = res.results[c]
        b, half = c // 2, c % 2
        qs = slice(half * SQ, (half + 1) * SQ)
        out[b, qs, :] = r["outp"]
        attn[b, :, qs, :] = np.swapaxes(r["attnT"], 1, 2)
    return out, attn
